# revision 5
# baseline (speedup 1.0000x reference)
"""Multi-head cross-attention TRN2 kernel, v3.

N=4096, D=256, H=4, K=16. Data-parallel over 8 NeuronCores: each core owns
R=512 query rows, key_value + weights replicated. No collectives.

Differences vs the v2 baseline (kernel.py):
  * Stage = (head-pair, 128-key chunk) instead of (head, 256-key pair).
    The two S matmuls of a stage use DIFFERENT PE row groups (heads at
    32-aligned partition bases), and the two AV matmuls use DIFFERENT col
    groups, so each pair runs CONCURRENTLY in the 128x128 array (row/col
    tile_position packing; K=16 and M=32 use 1/4 of the array each).
    PE busy drops ~61us -> ~34us.
  * Heads live at partition base 32h in ONE qt/kht tile (Q3=True) or two
    pair tiles at {0,32} (Q3=False fallback if quadrant 3 is buggy).
  * v_aug groups padded to 32 cols/head (ones slot + 16 V cols + 15 zeros)
    so AV writes full 32-row groups; the whole epilogue is then batched:
    ONE reciprocal + ONE rb-broadcast group + ONE copy + ONE hn multiply
    covers all 4 heads (DVE cost is free-dim-only). DVE epilogue ~10.5us
    -> ~2us.
  * Every 5th stage's softmax exp runs ENTIRELY on DVE as a Schraudolph
    fast-exp (int16 tile bitcast to bf16 for the AV matmul); the other
    stages use exact exp on ACT. Whole-stage offload keeps each es tile
    single-engine (a mixed ACT/DVE tile false-WAWs via bitcast and
    serializes the pipeline). Rel err ~5.4e-3.
  * s_psum rotates 3-deep (hp/mpsum single-buffered), out-store DMAs ride
    the SP queue, and the epilogue multiplies hp_sb(SBUF) by rb straight
    from PSUM — all three relieve queue-head serialization.

Schedule: same ACT-paced flat pipeline as v2 (filler/pri queues pumped
into the PE/DVE slack between stages; next call's prep double-buffered).

Everything fed to the PE is bf16 (cast on host); accumulation fp32;
output fp32.
"""
from collections import deque

import numpy as np
import ml_dtypes

import concourse.bass as bass
from concourse import bacc
import concourse.mybir as mybir
import concourse.tile as tile
from concourse.bass_utils import run_bass_kernel_spmd

N, D, H, K = 4096, 256, 4, 16
NCORES = 8
R = N // NCORES          # 512 query rows per core
NCHUNK = N // 128        # 32 key chunks
F32 = mybir.dt.float32
BF16 = mybir.dt.bfloat16
I16 = mybir.dt.int16
EXPF = mybir.ActivationFunctionType.Exp
BF = ml_dtypes.bfloat16

TRACE = False
LAST_RESULTS = None

# --- tunables ---
Q3 = True                # heads at bases {0,32,64,96} in single tiles; if the
                         # quadrant-3 HW bug bites, flip to pair tiles {0,32}
STRIP_X = 0              # exp columns per stage offloaded to DVE (Schraudolph
                         # strip) — superseded by DVE_EVERY (the strip write
                         # false-WAWs against ACT's exp in the same tile)
SKIP_CONST_MEMSETS = True  # v_aug pads/ones are identical every iteration and
                         # the consts pool double-buffers tag-stably, so after
                         # the first two emissions the memsets are redundant
O_PAIR = True            # two W_o query-chunks share one PSUM tile + one copy
ABLATE = ""              # timing-only ablations (output garbage): comma-set of
                         # noact,nostrip,nos,noav,noprep,noepi
SPSUM3 = True            # s_psum 3-deep rotation (6 banks) + single-buffered
                         # hp/mpsum; deeper pipeline tolerance vs prep/epilogue
                         # serialization
DVE_EVERY = 5            # every k-th stage's exp runs ENTIRELY on DVE
                         # (Schraudolph into its own int16 tile -> no
                         # mixed-engine es tile -> no false WAW edge);
                         # use with STRIP_X=0
MPSUM256 = False         # mpsum tiles [128,256] x2 bufs in ONE bank (prep
                         # matmuls split 256-wide): with SPSUM3 this restores
                         # double-buffered prep so PE never head-blocks on a
                         # queued DVE copy
O_ON_SP = True           # out-store DMAs ride the SP queue instead of ACT
                         # (ACT-queue enqueues wait o_sb copies and block exp)
EPI_HPSB = True          # epilogue: copy hp->SBUF in parallel with recip/rb,
                         # then hn = hp_sb * rb_psum(PSUM); one fewer
                         # cross-engine hop on the iteration-boundary chain
FILL_NS = 200
PUMP_CAP = 260.0
PREP_STAGE = 8
AV_LAG = 3
ES_BUFS = 8

SCH_A = 0.25 * 128.0 / float(np.log(2.0))
SCH_B = 127.0 * 128.0 - 128.0 * 0.043


def _build(repeats=1):
    nc = bacc.Bacc()
    q = nc.declare_dram_parameter("q", [R, D], BF16, isOutput=False)
    kv = nc.declare_dram_parameter("kv", [N, D], BF16, isOutput=False)
    # wqkv blob [768, 128]: rows 384t..384t+384 = d-half t, cols:
    #   0:128 wq_pad | 128:256 wk_pad | 256:384 wv_pad  (transposed on DMA)
    # wq/wk_pad: head h at cols 32h..32h+16.  wv_pad: head h V at cols
    # 32h+1..32h+17 (col 32h is the ones slot, filled on device).
    wqkv = nc.declare_dram_parameter("wqkv", [768, 128], BF16, isOutput=False)
    # wo blob [128, 256]: rows 32h+1+k = W_o[h*16+k]; rows 32h and
    # 32h+17..32h+32 zero.
    wo = nc.declare_dram_parameter("wo", [128, D], BF16, isOutput=False)
    out = nc.declare_dram_parameter("out", [R, D], F32, isOutput=True)

    with tile.TileContext(nc) as tc:
        with (
            tc.tile_pool(name="persist", bufs=1) as persist,
            tc.tile_pool(name="consts", bufs=2) as consts,
            tc.tile_pool(name="es", bufs=ES_BUFS) as espool,
            tc.tile_pool(name="sbops", bufs=3) as sbops,
            tc.tile_pool(name="spsum", bufs=3 if SPSUM3 else 2,
                         space="PSUM") as spsum,
            tc.tile_pool(name="hpsum", bufs=1 if SPSUM3 else 2,
                         space="PSUM") as hpsum,
            tc.tile_pool(name="mpsum", bufs=1 if SPSUM3 else 2,
                         space="PSUM") as mpsum,
        ):
            # ones_rb const (input-independent): row 32h cols 0:17 = 1.
            ones_rb = persist.tile([128, 32], BF16, name="ones_rb")
            nc.vector.memset(ones_rb, 0.0)
            for h in range(H):
                nc.vector.memset(ones_rb[32 * h:32 * h + 1, 0:17], 1.0)

            pipe = _Pipeline(nc, consts, espool, sbops, spsum, hpsum, mpsum,
                             q, kv, wqkv, wo, out, ones_rb)
            ctx = None
            for rep in range(repeats):
                if ctx is None:
                    ctx = pipe.emit_prep_inline()
                ctx_next = pipe.run_iteration(ctx, prep_next=(rep < repeats - 1))
                ctx = ctx_next
            pipe.drain()

    nc.finalize()
    return nc


class _Pipeline:
    def __init__(self, nc, consts, espool, sbops, spsum, hpsum, mpsum,
                 q, kv, wqkv, wo, out, ones_rb):
        self.nc = nc
        self.consts = consts
        self.espool = espool
        self.sbops = sbops
        self.spsum = spsum
        self.hpsum = hpsum
        self.mpsum = mpsum
        self.q, self.kv, self.wqkv, self.wo, self.out = q, kv, wqkv, wo, out
        self.ones_rb = ones_rb
        self.filler = deque()       # (pe_cost_ns, closure) — bulk prep work
        self.pri = deque()          # latency-critical (epilogue, W_o/out)
        self.spent = 0.0
        self.allowance = 0.0
        self.prep_count = 0

    def pump(self, budget_ns):
        self.allowance = min(self.allowance + budget_ns, self.spent + PUMP_CAP)
        while (self.pri or self.filler) and self.spent <= self.allowance:
            q = self.pri if self.pri else self.filler
            cost, fn = q.popleft()
            fn()
            self.spent += cost

    def drain(self):
        while self.pri or self.filler:
            q = self.pri if self.pri else self.filler
            cost, fn = q.popleft()
            fn()

    # ---------- per-iteration resource prep (DMAs + projections) ----------

    def prep_closures(self):
        nc = self.nc
        consts = self.consts
        ctx = {}
        cls = []

        def dmas():
            wqkv_sb = consts.tile([128, 768], BF16, tag="wqkv_sb", name="wqkv_sb")
            nc.sync.dma_start(out=wqkv_sb, in_=self.wqkv[:, :], transpose=True)
            qt0 = consts.tile([128, R], BF16, tag="qt0", name="qt0")
            qt1 = consts.tile([128, R], BF16, tag="qt1", name="qt1")
            kt0 = consts.tile([128, N], BF16, tag="kt0", name="kt0")
            kt1 = consts.tile([128, N], BF16, tag="kt1", name="kt1")
            nc.sync.dma_start(out=qt0, in_=self.q[:, 0:128], transpose=True)
            nc.sync.dma_start(out=qt1, in_=self.q[:, 128:256], transpose=True)
            for j in range(N // 512):
                sl = slice(512 * j, 512 * (j + 1))
                nc.sync.dma_start(out=kt0[:, sl], in_=self.kv[sl, 0:128],
                                  transpose=True)
                nc.sync.dma_start(out=kt1[:, sl], in_=self.kv[sl, 128:256],
                                  transpose=True)
            wo_sb = consts.tile([128, D], BF16, tag="wo_sb", name="wo_sb")
            nc.sync.dma_start(out=wo_sb, in_=self.wo[:, :])
            ctx.update(wqkv_sb=wqkv_sb, qt0=qt0, qt1=qt1, kt0=kt0, kt1=kt1,
                       wo_sb=wo_sb)

        fresh = (not SKIP_CONST_MEMSETS) or self.prep_count < 2
        self.prep_count += 1

        def v_zero():
            # pads must be zero: AV reads full 32-col groups. The pads and
            # ones column are identical every iteration, and the 2-buf pool
            # rotates tag-stably, so only the first two emissions memset.
            v_aug = consts.tile([128, 32 * NCHUNK * H], BF16, tag="v_aug",
                                name="v_aug")
            if fresh:
                nc.vector.memset(v_aug, 0.0)
                ones_v = v_aug[:].rearrange("p (c h s) -> p c h s",
                                            c=NCHUNK, s=32)[:, :, :, 0:1]
                nc.vector.memset(ones_v, 1.0)
            ctx.update(v_aug=v_aug)

        MW = 256 if MPSUM256 else 512

        def qt_proj():
            wqkv_sb, qt0, qt1 = ctx["wqkv_sb"], ctx["qt0"], ctx["qt1"]
            qt = consts.tile([128, R], BF16, tag="qt", name="qt")
            for w in range(0, R, MW):
                qt_psum = self.mpsum.tile([128, MW], F32, tag="m", name="m")
                nc.tensor.matmul(qt_psum[:], wqkv_sb[:, 0:128],
                                 qt0[:, w:w + MW], start=True, stop=False)
                nc.tensor.matmul(qt_psum[:], wqkv_sb[:, 384:512],
                                 qt1[:, w:w + MW], start=False, stop=True)
                nc.vector.tensor_copy(qt[:, w:w + MW], qt_psum[:])
            kht = consts.tile([128, N], BF16, tag="kht", name="kht")
            ctx.update(qt=qt, kht=kht)

        def kh_a(j, w):
            def f():
                kh_psum = self.mpsum.tile([128, MW], F32, tag="m", name="m")
                nc.tensor.matmul(kh_psum[:], ctx["wqkv_sb"][:, 128:256],
                                 ctx["kt0"][:, w:w + MW],
                                 start=True, stop=False)
                ctx["kh_psum"] = kh_psum
            return f

        def kh_b(j, w):
            def f():
                kh_psum = ctx.pop("kh_psum")
                nc.tensor.matmul(kh_psum[:], ctx["wqkv_sb"][:, 512:640],
                                 ctx["kt1"][:, w:w + MW],
                                 start=False, stop=True)
                nc.vector.tensor_copy(ctx["kht"][:, w:w + MW], kh_psum[:])
            return f

        def v_mm(i, half):
            # chunk c = 2*i' + half; two chunks share one [128, 256] psum and
            # ONE strided copy of their V columns into v_aug
            def f():
                wqkv_sb, v_aug = ctx["wqkv_sb"], ctx["v_aug"]
                if half == 0:
                    ctx["v_psum"] = self.mpsum.tile([128, 256], F32,
                                                    tag="m", name="m")
                v_psum = ctx["v_psum"]
                vp = v_psum[:, 128 * half:128 * (half + 1)]
                nc.tensor.matmul(vp, ctx["kt0"][:, 128 * i:128 * (i + 1)],
                                 wqkv_sb[:, 256:384], start=True, stop=False)
                nc.tensor.matmul(vp, ctx["kt1"][:, 128 * i:128 * (i + 1)],
                                 wqkv_sb[:, 640:768], start=False, stop=True)
                if half == 1:
                    del ctx["v_psum"]
                    i0 = i - 1
                    vsrc = v_psum[:].rearrange("p (c h s) -> p c h s",
                                               c=2, s=32)[:, :, :, 1:17]
                    vdst = v_aug[:, 128 * i0:128 * (i0 + 2)].rearrange(
                        "p (c h s) -> p c h s", c=2, s=32)[:, :, :, 1:17]
                    nc.vector.tensor_copy(vdst, vsrc)
            return f

        cls.append((60.0, dmas))
        cls.append((60.0, v_zero))
        cls.append((500.0, qt_proj))
        kcost = 220.0 * MW / 512
        for j in range(N // 512):
            for w in range(512 * j, 512 * (j + 1), MW):
                cls.append((kcost, kh_a(j, w)))
                cls.append((kcost, kh_b(j, w)))
            for i in range(4 * j, 4 * j + 4):
                cls.append((75.0, v_mm(i, i % 2)))
        return ctx, cls

    def emit_prep_inline(self):
        ctx, cls = self.prep_closures()
        for _, fn in cls:
            fn()
        return ctx

    # ---------- one attention iteration ----------

    def run_iteration(self, ctx, prep_next):
        nc = self.nc
        es_tiles = {}
        hp_box = {}

        abl = set(ABLATE.split(",")) if ABLATE else set()

        def s_stage(P, c):
            kht, qt = ctx["kht"], ctx["qt"]
            b0 = 64 * P
            s_psum = self.spsum.tile([128, 1024], F32, tag="s", name="s")
            ck = slice(128 * c, 128 * (c + 1))
            if "nos" not in abl:
                nc.tensor.matmul(s_psum[:, 0:512],
                                 kht[b0:b0 + 16, ck], qt[b0:b0 + 16, :],
                                 start=True, stop=True, tile_position=(b0, 0))
                nc.tensor.matmul(s_psum[:, 512:1024],
                                 kht[b0 + 32:b0 + 48, ck],
                                 qt[b0 + 32:b0 + 48, :],
                                 start=True, stop=True,
                                 tile_position=(b0 + 32, 0))
            t = 32 * P + c
            if DVE_EVERY and t % DVE_EVERY == DVE_EVERY - 1:
                # whole-stage Schraudolph exp on DVE, own int16 tile: no
                # mixed-engine tile, so no false WAW against ACT
                esi = self.espool.tile([128, 1024], I16, tag="esi", name="esi")
                if "nostrip" not in abl:
                    nc.vector.tensor_scalar(esi[:], s_psum[:], SCH_A, SCH_B,
                                            op0=mybir.AluOpType.mult,
                                            op1=mybir.AluOpType.add)
                es_tiles[(P, c)] = esi.bitcast(BF16)
                return
            es = self.espool.tile([128, 1024], BF16, tag="es", name="es")
            lo = 1024 - STRIP_X if STRIP_X else 1024
            if "noact" not in abl and lo:
                nc.scalar.activation(es[:, 0:lo], s_psum[:, 0:lo],
                                     EXPF, scale=0.25)
            if STRIP_X and "nostrip" not in abl:
                # NOTE: this write false-WAWs against the exp above (bitcast
                # defeats subtile ranges) — serializes ACT->DVE per stage.
                # Prefer DVE_EVERY whole-stage offload instead.
                nc.vector.tensor_scalar(es.bitcast(I16)[:, lo:1024],
                                        s_psum[:, lo:1024], SCH_A, SCH_B,
                                        op0=mybir.AluOpType.mult,
                                        op1=mybir.AluOpType.add)
            es_tiles[(P, c)] = es

        def av_stage(P, c):
            if "noav" in abl:
                es_tiles.pop((P, c))
                return
            if P == 0 and c == 0:
                hp_box["hp"] = self.hpsum.tile([128, R], F32, tag="heads",
                                               name="heads")
            hp = hp_box["hp"]
            es = es_tiles.pop((P, c))
            v_aug = ctx["v_aug"]
            g = 128 * c + 64 * P
            nc.tensor.matmul(hp[64 * P:64 * P + 32, :],
                             v_aug[:, g:g + 32], es[:, 0:512],
                             start=(c == 0), stop=(c == NCHUNK - 1),
                             tile_position=(0, 64 * P))
            nc.tensor.matmul(hp[64 * P + 32:64 * P + 64, :],
                             v_aug[:, g + 32:g + 64], es[:, 512:1024],
                             start=(c == 0), stop=(c == NCHUNK - 1),
                             tile_position=(0, 64 * P + 32))

        def epilogue():
            hp = hp_box.pop("hp")
            st = {}

            def ep_recip():
                recipb = self.sbops.tile([128, R], BF16, tag="recipb",
                                         name="recipb")
                with nc.allow_low_precision(
                        reason="recip feeds a bf16 matmul operand anyway"):
                    nc.vector.reciprocal(recipb[0:97, :], hp[0:97, :])
                st["recipb"] = recipb

            def ep_rb():
                ps = []
                for w in range(0, R, 256 if MPSUM256 else 512):
                    mw = 256 if MPSUM256 else 512
                    rb_psum = self.mpsum.tile([128, mw], F32, tag="m", name="m")
                    for h in range(H):
                        nc.tensor.matmul(rb_psum[32 * h:32 * h + 32, :],
                                         self.ones_rb[32 * h:32 * h + 1, :],
                                         st["recipb"][32 * h:32 * h + 1,
                                                      w:w + mw],
                                         start=True, stop=True,
                                         tile_position=(32 * h, 32 * h))
                    ps.append((w, mw, rb_psum))
                st["rb_psum"] = ps

            def ep_hpsb():
                hp_sb = self.sbops.tile([128, R], BF16, tag="hp_sb",
                                        name="hp_sb")
                with nc.allow_low_precision(
                        reason="hp feeds a bf16 matmul operand anyway"):
                    nc.vector.tensor_copy(hp_sb[:], hp[:])
                st["hp_sb"] = hp_sb

            def ep_rbcopy():
                rb_sb = self.sbops.tile([128, R], F32, tag="rb_sb", name="rb_sb")
                for w, mw, rb_psum in st["rb_psum"]:
                    nc.vector.tensor_copy(rb_sb[:, w:w + mw], rb_psum[:])
                st["rb"] = rb_sb

            def ep_mul():
                hn = self.consts.tile([128, R], BF16, tag="hn", name="hn")
                if EPI_HPSB:
                    for w, mw, rb_psum in st["rb_psum"]:
                        nc.vector.tensor_mul(hn[:, w:w + mw],
                                             st["hp_sb"][:, w:w + mw],
                                             rb_psum[:])
                else:
                    nc.vector.tensor_mul(hn[:], hp[:], st["rb"][:])
                ctx["hn"] = hn

            if EPI_HPSB:
                return [(60.0, ep_hpsb), (60.0, ep_recip), (260.0, ep_rb),
                        (60.0, ep_mul)]
            return [(60.0, ep_recip), (260.0, ep_rb), (60.0, ep_rbcopy),
                    (60.0, ep_mul)]

        o_dma = (nc.sync.dma_start if O_ON_SP else nc.scalar.dma_start)

        def o_chunk(c):
            def f():
                cs = slice(128 * c, 128 * (c + 1))
                hn, wo_sb = ctx["hn"], ctx["wo_sb"]
                o_psum = self.mpsum.tile([128, D], F32, tag="m", name="m")
                nc.tensor.matmul(o_psum[:], hn[:, cs], wo_sb[:],
                                 start=True, stop=True)
                o_sb = self.sbops.tile([128, D], F32, tag="o_sb", name="o_sb",
                                       bufs=2)
                nc.vector.tensor_copy(o_sb[:], o_psum[:])
                o_dma(out=self.out[cs, :], in_=o_sb[:])
            return f

        def o_pair(c):
            # chunks c, c+1 share one [128, 512] psum + ONE copy
            def f():
                hn, wo_sb = ctx["hn"], ctx["wo_sb"]
                o_psum = self.mpsum.tile([128, 2 * D], F32, tag="m", name="m")
                nc.tensor.matmul(o_psum[:, 0:D], hn[:, 128 * c:128 * (c + 1)],
                                 wo_sb[:], start=True, stop=True)
                nc.tensor.matmul(o_psum[:, D:2 * D],
                                 hn[:, 128 * (c + 1):128 * (c + 2)],
                                 wo_sb[:], start=True, stop=True)
                o_sb = self.sbops.tile([128, 2 * D], F32, tag="o_sb",
                                       name="o_sb", bufs=2)
                nc.vector.tensor_copy(o_sb[:], o_psum[:])
                o_dma(out=self.out[128 * c:128 * (c + 1), :],
                      in_=o_sb[:, 0:D])
                o_dma(out=self.out[128 * (c + 1):128 * (c + 2), :],
                      in_=o_sb[:, D:2 * D])
            return f

        ctx_next, prep = (self.prep_closures() if prep_next else (None, []))
        prep = deque(prep)

        stages = [(P, c) for P in range(2) for c in range(NCHUNK)]

        def retire(t):
            av_stage(*stages[t])
            if t == len(stages) - 1:
                if "noepi" in abl or "noav" in abl:
                    hp_box.clear()
                    return
                self.pri.extend(epilogue())
                if O_PAIR and not MPSUM256:
                    for c in range(0, R // 128, 2):
                        self.pri.append((500.0, o_pair(c)))
                else:
                    for c in range(R // 128):
                        self.pri.append((280.0, o_chunk(c)))

        for t, (P, c) in enumerate(stages):
            s_stage(P, c)
            if prep and t == PREP_STAGE:
                self.filler.extend(prep)
                prep.clear()
            self.pump(FILL_NS)
            if t >= AV_LAG:
                retire(t - AV_LAG)
        for t in range(len(stages) - AV_LAG, len(stages)):
            retire(t)
        if prep:
            self.filler.extend(prep)
        return ctx_next


_NC_CACHE = None


def _host_in_maps(query, key_value, W_q, W_k, W_v, W_o):
    q_bf = np.ascontiguousarray(query.astype(BF))
    kv_bf = np.ascontiguousarray(key_value.astype(BF))
    wqkv_h = np.zeros((D, 384), dtype=BF)
    wqt = np.transpose(W_q, (1, 0, 2))  # [D, H, K]
    wkt = np.transpose(W_k, (1, 0, 2))
    wvt = np.transpose(W_v, (1, 0, 2))
    for h in range(H):
        c0 = 32 * h
        wqkv_h[:, c0:c0 + K] = wqt[:, h, :].astype(BF)
        wqkv_h[:, 128 + c0:128 + c0 + K] = wkt[:, h, :].astype(BF)
        wqkv_h[:, 256 + c0 + 1:256 + c0 + 1 + K] = wvt[:, h, :].astype(BF)
    wqkv_hh = np.ascontiguousarray(np.concatenate(
        [wqkv_h[0:128].T, wqkv_h[128:256].T], axis=0))
    wo_h = np.zeros((128, D), dtype=BF)
    wo_r = W_o.reshape(H, K, D)
    for h in range(H):
        wo_h[32 * h + 1:32 * h + 1 + K, :] = wo_r[h].astype(BF)
    return [{"q": q_bf[c * R:(c + 1) * R], "kv": kv_bf, "wqkv": wqkv_hh,
             "wo": wo_h}
            for c in range(NCORES)]


def kernel(query, key_value, W_q, W_k, W_v, W_o):
    global _NC_CACHE, LAST_RESULTS
    if _NC_CACHE is None:
        _NC_CACHE = _build()
    nc = _NC_CACHE
    in_maps = _host_in_maps(query, key_value, W_q, W_k, W_v, W_o)
    res = run_bass_kernel_spmd(nc, in_maps, list(range(NCORES)), trace=TRACE)
    LAST_RESULTS = res
    return np.concatenate([res.results[c]["out"] for c in range(NCORES)], axis=0)
